# revision 36
# baseline (speedup 1.0000x reference)
"""Trainium2 Bass kernel for an attention-LSTM decoder (Bahdanau attention +
LSTM cell + generator head), data-parallel over 8 NeuronCores.

Shapes (hardcoded): B=1024, T=64, D=256, H=256, C=96, steps S=26.
Each core handles 128 batch rows.

Device layouts (per core, b = 128):
  - score chain runs "transposed": partitions = hidden dim tiles (2 x 128),
    free = (b, t) flat b-major.
  - softmax/context run natural: partitions = b, free = t / d.
  - LSTM/gates run transposed: gatesT [4H -> 8 tiles of 128, b].
Matmul operands are fp16 (full-rate PE streaming, 10-bit mantissa); PSUM
accumulation is fp32; the recurrent c state is fp32.

Host-side prep (numpy): fp16 casts, batch_H transpose for the projection
matmul, one-hot text encoding, b_lstm folded into the one-hot weight rows
(valid because one-hot rows sum to 1), bg added to the final output.
"""

import sys

for _p in ("/opt/trn_rl_repo",):
    if _p not in sys.path:
        sys.path.insert(0, _p)

import numpy as np

import concourse.bass as bass
import concourse.tile as tile
from concourse import mybir
from concourse.bass_utils import run_bass_kernel_spmd
from concourse.masks import make_identity

dt = mybir.dt
AF = mybir.ActivationFunctionType
ALU = mybir.AluOpType

NCORES = 8
B, T, D, H, C = 1024, 64, 256, 256, 96
S = 26  # num steps = batch_max_length + 1
BC = B // NCORES  # 128 batch rows per core
KT = 2  # 256 = 2 x 128 tiles for d/h contraction
GT = 8  # 4H = 1024 = 8 m-tiles of 128
TB = BC * T  # 8192, flat (b, t) b-major
NCHUNK = 512  # psum-bank-limited matmul N
ZCH = 4096  # z/tanh chunk along flat (b, t): 64 b x 64 t
COLG = (0, 32, 64)  # PE column groups for e-matmul output spreading

_CACHE = {}


def _ap_bcast_t(base, nb, nt):
    """AP reading base [128, nb] broadcast over an inner t dim of size nt."""
    return bass.AP(tensor=base.tensor, offset=base.offset,
                   ap=[base.ap[0], [base.ap[-1][0], nb], [0, nt]])


def _split_excess_waits(nc, max_waits=1):
    """This container's walrus rejects instructions carrying more than
    ~max_waits semaphore waits ("Too many sync wait commands"). Hoist excess
    waits onto InstNoOp instructions inserted just before, on the same engine
    (per-engine program order makes this semantics-preserving)."""
    nid = [0]
    for f in nc.m.functions:
        for blk in f.blocks:
            insts = blk.instructions
            out = []
            changed = False
            for ins in insts:
                si = ins.sync_info
                ow = list(si.on_wait) if si is not None and si.on_wait else []
                if len(ow) > max_waits:
                    changed = True
                    while len(ow) > max_waits:
                        take, ow = ow[:max_waits], ow[max_waits:]
                        nid[0] += 1
                        nop = mybir.InstNoOp(
                            name=f"WSPLIT-{nid[0]}", engine=ins.engine,
                            sync_info=mybir.SyncInfo(on_wait=take,
                                                     on_update=[]))
                        nc.register_instruction(nop, overwrite=True)
                        out.append(nop)
                    ins.sync_info = mybir.SyncInfo(
                        on_wait=ow, on_update=list(si.on_update or []))
                out.append(ins)
            if changed:
                blk.instructions = out


def _build():
    nc = bass.Bass("TRN2", target_bir_lowering=False)
    f16, f32 = dt.float16, dt.float32

    h_nat_d = nc.declare_dram_parameter("h_nat", [BC, T, D], f16, isOutput=False)
    h_t_d = nc.declare_dram_parameter("h_t", [D, BC, T], f16, isOutput=False)
    oneh_d = nc.declare_dram_parameter("onehot", [C, S, BC], f16, isOutput=False)
    wi2h_d = nc.declare_dram_parameter("wi2h", [D, H], f16, isOutput=False)
    wh2h_d = nc.declare_dram_parameter("wh2h", [H, H], f16, isOutput=False)
    bh2h_d = nc.declare_dram_parameter("bh2hT", [H, 1], f32, isOutput=False)
    wsc_d = nc.declare_dram_parameter("wsc", [H, 32], f16, isOutput=False)
    wxc_d = nc.declare_dram_parameter("wxc", [D, 4 * H], f16, isOutput=False)
    wxo_d = nc.declare_dram_parameter("wxo", [C, 4 * H], f16, isOutput=False)
    wh_d = nc.declare_dram_parameter("wh", [H, 4 * H], f16, isOutput=False)
    wg_d = nc.declare_dram_parameter("wg", [H, C], f16, isOutput=False)
    probs_d = nc.declare_dram_parameter("probsT", [C, S, BC], f32, isOutput=True)

    with tile.TileContext(nc) as tc:
        import contextlib
        ctx = contextlib.ExitStack()
        with ctx:
            singles = ctx.enter_context(tc.tile_pool(name="singles", bufs=1))
            psA = ctx.enter_context(tc.tile_pool(name="psA", bufs=1, space="PSUM"))
            psB = ctx.enter_context(tc.tile_pool(name="psB", bufs=1, space="PSUM"))

            # ---- persistent SBUF state ----
            h_nat = singles.tile([BC, T, D], f16)
            projT = singles.tile([128, KT, BC, T], f16)  # [h', m, b, t]
            hidT = singles.tile([128, KT, S + 1, BC], f16)  # h states, slot 0 = 0
            oneh = singles.tile([C, S, BC], f16)
            wi2h = singles.tile([128, KT, H], f16)
            wh2h = singles.tile([128, KT, H], f16)
            bh2hT = singles.tile([128, KT, 1], f32)
            wsc = singles.tile([128, KT, 32], f16)
            wxc = singles.tile([128, KT, 4 * H], f16)
            wxo = singles.tile([C, 4 * H], f16)
            wh = singles.tile([128, KT, 4 * H], f16)
            wg = singles.tile([128, KT, C], f16)
            ident = singles.tile([128, 128], f16)
            cT = singles.tile([BC, H], f16)

            # ---- load everything ----
            nc.sync.dma_start(out=h_nat, in_=h_nat_d[:])
            nc.sync.dma_start(out=oneh, in_=oneh_d[:])
            nc.sync.dma_start(
                out=wi2h, in_=wi2h_d[:].rearrange("(k p) h -> p k h", p=128))
            nc.sync.dma_start(
                out=wh2h, in_=wh2h_d[:].rearrange("(k p) h -> p k h", p=128))
            nc.sync.dma_start(
                out=bh2hT, in_=bh2h_d[:].rearrange("(k p) o -> p k o", p=128))
            nc.sync.dma_start(
                out=wsc, in_=wsc_d[:].rearrange("(k p) o -> p k o", p=128))
            nc.sync.dma_start(
                out=wxc, in_=wxc_d[:].rearrange("(k p) g -> p k g", p=128))
            nc.sync.dma_start(out=wxo, in_=wxo_d[:])
            nc.sync.dma_start(
                out=wh, in_=wh_d[:].rearrange("(k p) g -> p k g", p=128))
            nc.sync.dma_start(
                out=wg, in_=wg_d[:].rearrange("(k p) c -> p k c", p=128))
            make_identity(nc, ident)
            nc.vector.memset(hidT[:, :, 0, :], 0.0)
            nc.vector.memset(cT, 0.0)

            # ---- precompute projT = (batch_H @ Wi2h)^T : [h', m, (b t)] ----
            projT_f = projT[:].rearrange("p m b t -> p m (b t)")
            with tc.tile_pool(name="ht", bufs=1) as ht_pool:
                h_tt = ht_pool.tile([128, KT, BC, T], f16)
                nc.sync.dma_start(
                    out=h_tt,
                    in_=h_t_d[:].rearrange("(k p) b t -> p k b t", p=128))
                h_tt_f = h_tt[:].rearrange("p k b t -> p k (b t)")
                for m in range(KT):
                    for c in range(TB // NCHUNK):
                        ps = psA.tile([128, 6, NCHUNK], f32, tag="e")
                        sl = slice(c * NCHUNK, (c + 1) * NCHUNK)
                        for k in range(KT):
                            nc.tensor.matmul(
                                ps[:, 0, :], wi2h[:, k, m * 128:(m + 1) * 128],
                                h_tt_f[:, k, sl], start=(k == 0),
                                stop=(k == KT - 1))
                        eng = nc.scalar if (c % 2 == 0) else nc.vector
                        if eng is nc.scalar:
                            eng.copy(out=projT_f[:, m, sl], in_=ps[:, 0, :])
                        else:
                            eng.tensor_copy(projT_f[:, m, sl], ps[:, 0, :])

            work = ctx.enter_context(tc.tile_pool(name="work", bufs=2))
            small = ctx.enter_context(tc.tile_pool(name="small", bufs=2))
            big1 = ctx.enter_context(tc.tile_pool(name="big1", bufs=1))
            ahp = ctx.enter_context(tc.tile_pool(name="ahp", bufs=10))

            # ---- decode steps ----
            for s in range(S):
                # hp = h @ Wh2h + bh2h, transposed [h', m, b]
                ps_hp = psB.tile([BC, KT, 128], f32, tag="misc")
                for m in range(KT):
                    for k in range(KT):
                        nc.tensor.matmul(
                            ps_hp[:, m, :], wh2h[:, k, m * 128:(m + 1) * 128],
                            hidT[:, k, s, :], start=(k == 0), stop=(k == KT - 1))
                hpT = small.tile([128, KT, 128], f16)
                for m in range(KT):
                    nc.vector.tensor_scalar(
                        out=hpT[:, m, :], in0=ps_hp[:, m, :],
                        scalar1=bh2hT[:, m, :], scalar2=None, op0=ALU.add)
                # replicate hp 8x in an inner dim so the z-add's broadcast AP
                # ends with a step-1 dim (keeps DVE 2x mode); per-m so the
                # z-pipeline fills sooner
                hpR = small.tile([128, KT, 128, 8], f16, tag="hpR")
                for m in range(KT):
                    base = hpT[:, m, :]
                    nc.vector.tensor_copy(
                        hpR[:, m, :, :],
                        bass.AP(tensor=base.tensor, offset=base.offset,
                                ap=[base.ap[0], [1, 128], [0, 8]]))

                # z = projT + hp (bcast over t); tanh; e = w . tanh
                ps_e = psA.tile([128, 6, NCHUNK], f32, tag="e")
                for m in range(KT):
                    cg = 0
                    b0 = 0
                    for nbz in (8, 24, 32, 64):
                        bsl = slice(b0, b0 + nbz)
                        gp = (m == KT - 1 and b0 == 64)
                        # the last chunk of m=1 runs on gpsimd (plain TT add)
                        # from a dedicated tile so it isn't starved by the
                        # z-slot rotation
                        z = work.tile([128, 64, T], f16,
                                      tag=("zgp" if gp else "z"))
                        hb = hpR[:, m, b0:, :]
                        zeng = nc.vector  # gpsimd TT crashed HW (NRT 101)
                        zeng.tensor_tensor(
                            out=z[:, :nbz, :].rearrange(
                                "p b (r i) -> p b r i", i=8),
                            in0=projT[:, m, bsl, :].rearrange(
                                "p b (r i) -> p b r i", i=8),
                            in1=bass.AP(tensor=hb.tensor, offset=hb.offset,
                                        ap=[hb.ap[0], [hb.ap[1][0], nbz],
                                            [0, T // 8], [1, 8]]),
                            op=ALU.add)
                        th = work.tile([128, 64 * T], f16, tag="th")
                        nc.scalar.activation(
                            out=th[:, :nbz * T],
                            in_=z[:, :nbz, :].rearrange("p b t -> p (b t)"),
                            func=AF.Tanh)
                        for q in range(nbz * T // NCHUNK):
                            col = 32 * (cg // 6)
                            bank = cg % 6
                            nc.tensor.matmul(
                                ps_e[col:col + 32, bank, :], wsc[:, m, :],
                                th[:, q * NCHUNK:(q + 1) * NCHUNK],
                                start=(m == 0), stop=(m == KT - 1),
                                tile_position=(0, col),
                                skip_group_check=True)
                            cg += 1
                        b0 += nbz
                    assert b0 == BC

                # softmax over t (e bounded: no max subtraction needed).
                # Chunk cg lives replicated on partitions
                # [32*(cg//6), +32), bank cg%6. Valid rectangles:
                # [0:96, banks 0:4] and [0:64, banks 4:6].
                expa = big1.tile([128, 6, NCHUNK], f32, tag="expa")
                nc.scalar.activation(out=expa[0:96, 0:4, :],
                                     in_=ps_e[0:96, 0:4, :], func=AF.Exp)
                nc.scalar.activation(out=expa[0:64, 4:6, :],
                                     in_=ps_e[0:64, 4:6, :], func=AF.Exp)
                # unnormalized alpha scatter -> [128 b, 64 t]; chunk cg is
                # on partition-row 32*(cg//6), bank cg%6, so rows 0/32 cover
                # chunks 0..11 (-> b 0..96) and row 64 banks 0..3 -> b 96..128
                alpha = small.tile([BC, T], f32, tag="alpha")
                for row, nb in ((0, 6), (1, 6), (2, 4)):
                    er = expa[32 * row:32 * row + 1, 0:nb, :]
                    nc.sync.dma_start(
                        out=alpha[48 * row:48 * row + 8 * nb, :],
                        in_=bass.AP(tensor=er.tensor, offset=er.offset,
                                    ap=[er.ap[0], [NCHUNK, nb], [1, NCHUNK]]))
                # Z = sum_t exp in [b, t] layout; rzb = 1/Z  [128, 1]
                zsum = small.tile([BC, 1], f32, tag="zsum")
                nc.vector.reduce_sum(out=zsum, in_=alpha,
                                     axis=mybir.AxisListType.X)
                rzb = small.tile([BC, 1], f32, tag="rzb")
                nc.vector.reciprocal(out=rzb, in_=zsum)
                if s == 0:
                    _CACHE.setdefault("dbg", {})["alpha0"] = alpha.tensor.name
                    _CACHE["dbg"]["expa0"] = expa.tensor.name
                    _CACHE["dbg"]["hpT0"] = hpT.tensor.name
                # context = sum_t alpha~[:, t] * H[:, t, :] (unnormalized):
                # DVE makes alpha_t*H_t slices at 4x, PE accumulates them into
                # PSUM (fp32) via identity matmuls; the final copy rescales by
                # rzb = 1/Z.
                ps_cx = psB.tile([BC, D], f32, tag="misc")
                ahs = []
                for t in range(T):
                    ah = ahp.tile([BC, D], f16, tag="ah")
                    nc.vector.tensor_scalar(
                        out=ah, in0=h_nat[:, t, :], scalar1=alpha[:, t:t + 1],
                        scalar2=None, op0=ALU.mult)
                    ahs.append(ah)
                    if t % 8 == 7:
                        for j, a in enumerate(ahs):
                            tt = t - len(ahs) + 1 + j
                            nc.tensor.matmul(ps_cx, ident, a, start=(tt == 0),
                                             stop=(tt == T - 1))
                        ahs = []
                # gates partial (h and onehot contributions don't need ctx)
                ps_g = psA.tile([BC, 4 * H], f32, tag="e")
                for nchk in range(2):
                    nsl = slice(nchk * NCHUNK, (nchk + 1) * NCHUNK)
                    for k in range(KT):
                        nc.tensor.matmul(
                            ps_g[:, nsl], hidT[:, k, s, :], wh[:, k, nsl],
                            start=(k == 0), stop=False)
                    nc.tensor.matmul(
                        ps_g[:, nsl], oneh[:, s, :], wxo[:, nsl],
                        start=False, stop=False)
                ctxv = small.tile([BC, D], f16, tag="ctxv")
                nc.vector.tensor_scalar(
                    out=ctxv, in0=ps_cx, scalar1=rzb[:, 0:1], scalar2=None,
                    op0=ALU.mult)
                if s == 0:
                    _CACHE["dbg"]["ctx0"] = ctxv.tensor.name
                    _CACHE["dbg"]["projT"] = projT.tensor.name
                    _CACHE["dbg"]["hidT"] = hidT.tensor.name

                # transpose context -> [d', m, b]
                ps_ct = psB.tile([BC, KT, 128], f16, tag="misc")
                ctxT = small.tile([128, KT, 128], f16, tag="ctxT")
                for m in range(KT):
                    nc.tensor.transpose(
                        ps_ct[:, m, :], ctxv[:, m * 128:(m + 1) * 128], ident)
                    nc.vector.tensor_copy(ctxT[:, m, :], ps_ct[:, m, :])

                if s == 0:
                    _CACHE["dbg"]["ctxT0"] = ctxT.tensor.name
                # gates[b, 4H] = h.Wh + onehot.Wxo + ctx.Wxc with the
                # activations as the stationary operand (5 weight loads, 10
                # N=512 matmuls) instead of 40 small ones.
                for nchk in range(2):
                    nsl = slice(nchk * NCHUNK, (nchk + 1) * NCHUNK)
                    for k in range(KT):
                        nc.tensor.matmul(
                            ps_g[:, nsl], ctxT[:, k, :], wxc[:, k, nsl],
                            start=False, stop=(k == KT - 1))

                # LSTM pointwise in [b, g] layout. keras order i, f, g, o
                # (b_lstm folded into wxo host-side).
                sig_if = small.tile([BC, 2 * H], f16, tag="sig_if")
                tg = small.tile([BC, H], f16, tag="tg")
                sig_o = small.tile([BC, H], f16, tag="sig_o")
                nc.scalar.activation(out=sig_if, in_=ps_g[:, 0:2 * H],
                                     func=AF.Sigmoid)
                nc.scalar.activation(out=tg, in_=ps_g[:, 2 * H:3 * H],
                                     func=AF.Tanh)
                nc.scalar.activation(out=sig_o, in_=ps_g[:, 3 * H:4 * H],
                                     func=AF.Sigmoid)
                t1 = small.tile([BC, H], f16, tag="t1")
                t2 = small.tile([BC, H], f16, tag="t2")
                nc.vector.tensor_tensor(out=t1, in0=sig_if[:, H:2 * H],
                                        in1=cT, op=ALU.mult)
                nc.vector.tensor_tensor(out=t2, in0=sig_if[:, 0:H], in1=tg,
                                        op=ALU.mult)
                nc.vector.tensor_tensor(out=cT, in0=t1, in1=t2, op=ALU.add)
                tc_t = small.tile([BC, H], f16, tag="tc_t")
                nc.scalar.activation(out=tc_t, in_=cT, func=AF.Tanh)
                h_bd = small.tile([BC, H], f16, tag="h_bd")
                nc.vector.tensor_tensor(out=h_bd, in0=sig_o, in1=tc_t,
                                        op=ALU.mult)
                # transpose h back to [h', k, b] for the next step's matmuls
                ps_h = psB.tile([BC, KT, 128], f16, tag="misc")
                for k in range(KT):
                    nc.tensor.transpose(
                        ps_h[:, k, :], h_bd[:, k * 128:(k + 1) * 128], ident)
                    nc.vector.tensor_copy(hidT[:, k, s + 1, :], ps_h[:, k, :])

            # ---- generator: probsT = Wg^T . h_s for all steps ----
            hid_f = hidT[:].rearrange("p k s b -> p k (s b)")
            NS = S * BC  # 3328
            probs_sb = singles.tile([C, S, BC], f32)
            probs_f = probs_sb[:].rearrange("c s b -> c (s b)")
            pos = 0
            ci = 0
            while pos < NS:
                n = min(NCHUNK, NS - pos)
                ps_p = psB.tile([C, NCHUNK], f32, tag="misc")
                for k in range(KT):
                    nc.tensor.matmul(
                        ps_p[:, :n], wg[:, k, :], hid_f[:, k, BC + pos:BC + pos + n],
                        start=(k == 0), stop=(k == KT - 1))
                eng = nc.scalar if (ci % 2 == 0) else nc.vector
                if eng is nc.scalar:
                    eng.copy(out=probs_f[:, pos:pos + n], in_=ps_p[:, :n])
                else:
                    eng.tensor_copy(probs_f[:, pos:pos + n], ps_p[:, :n])
                pos += n
                ci += 1
            nc.sync.dma_start(out=probs_d[:], in_=probs_sb)

    _split_excess_waits(nc)
    return nc


def _get_module():
    if "nc" not in _CACHE:
        _CACHE["nc"] = _build()
    return _CACHE["nc"]


def build_in_maps(batch_H, text, batch_max_length, Wi2h, Wh2h, bh2h, w_score,
                  Wx, Wh, b_lstm, Wg, bg):
    batch_H = np.asarray(batch_H, dtype=np.float32)
    text = np.asarray(text)
    assert int(batch_max_length) + 1 == S
    assert batch_H.shape == (B, T, D)

    f16 = np.float16
    bh16 = batch_H.astype(f16)
    # one-hot text: [B, S, C] -> per-core [S, C, BC]
    oh = (text[:, :S, None] == np.arange(C)[None, None, :])

    Wx = np.asarray(Wx, np.float32)
    wxo_p = (Wx[D:D + C, :] + np.asarray(b_lstm, np.float32)[None, :]).astype(f16)
    weights = {
        "wi2h": np.ascontiguousarray(np.asarray(Wi2h, np.float32).astype(f16)),
        "wh2h": np.ascontiguousarray(np.asarray(Wh2h, np.float32).astype(f16)),
        "bh2hT": np.ascontiguousarray(
            np.asarray(bh2h, np.float32).reshape(H, 1)),
        "wsc": np.ascontiguousarray(np.tile(
            np.asarray(w_score, np.float32).reshape(H, 1), (1, 32)).astype(f16)),
        "wxc": np.ascontiguousarray(Wx[:D, :].astype(f16)),
        "wxo": np.ascontiguousarray(wxo_p),
        "wh": np.ascontiguousarray(np.asarray(Wh, np.float32).astype(f16)),
        "wg": np.ascontiguousarray(np.asarray(Wg, np.float32).astype(f16)),
    }

    in_maps = []
    for c in range(NCORES):
        bsl = slice(c * BC, (c + 1) * BC)
        in_maps.append({
            "h_nat": np.ascontiguousarray(bh16[bsl]),
            "h_t": np.ascontiguousarray(bh16[bsl].transpose(2, 0, 1)),
            "onehot": np.ascontiguousarray(
                oh[bsl].transpose(2, 1, 0).astype(f16)),
            **weights,
        })
    return in_maps


def kernel(**inputs):
    in_maps = build_in_maps(**inputs)
    bg = inputs["bg"]

    nc = _get_module()
    res = run_bass_kernel_spmd(nc, in_maps, list(range(NCORES)))

    out = np.empty((B, S, C), np.float32)
    for c in range(NCORES):
        out[c * BC:(c + 1) * BC] = res.results[c]["probsT"].transpose(2, 1, 0)
    out += np.asarray(bg, np.float32)[None, None, :]
    return out


if __name__ == "__main__":
    _build()
    print("build OK")


# revision 38
# speedup vs baseline: 1.2496x; 1.2496x over previous
"""Trainium2 Bass kernel for an attention-LSTM decoder (Bahdanau attention +
LSTM cell + generator head), data-parallel over 8 NeuronCores.

Shapes (hardcoded): B=1024, T=64, D=256, H=256, C=96, steps S=26.
Each core handles 128 batch rows.

Device layouts (per core, b = 128):
  - score chain runs "transposed": partitions = hidden dim tiles (2 x 128),
    free = (b, t) flat b-major.
  - softmax/context run natural: partitions = b, free = t / d.
  - LSTM/gates run transposed: gatesT [4H -> 8 tiles of 128, b].
Matmul operands are fp16 (full-rate PE streaming, 10-bit mantissa); PSUM
accumulation is fp32; the recurrent c state is fp32.

Host-side prep (numpy): fp16 casts, batch_H transpose for the projection
matmul, one-hot text encoding, b_lstm folded into the one-hot weight rows
(valid because one-hot rows sum to 1), bg added to the final output.
"""

import sys

for _p in ("/opt/trn_rl_repo", "/root/.axon_site/_ro/trn_rl_repo"):
    if _p not in sys.path:
        sys.path.insert(0, _p)

import numpy as np

import concourse.bass as bass
import concourse.tile as tile
from concourse import mybir
from concourse.bass_utils import run_bass_kernel_spmd
from concourse.masks import make_identity

dt = mybir.dt
AF = mybir.ActivationFunctionType
ALU = mybir.AluOpType

NCORES = 8
B, T, D, H, C = 1024, 64, 256, 256, 96
S = 26  # num steps = batch_max_length + 1
BC = B // NCORES  # 128 batch rows per core
KT = 2  # 256 = 2 x 128 tiles for d/h contraction
GT = 8  # 4H = 1024 = 8 m-tiles of 128
TB = BC * T  # 8192, flat (b, t) b-major
NCHUNK = 512  # psum-bank-limited matmul N
ZCH = 4096  # z/tanh chunk along flat (b, t): 64 b x 64 t
COLG = (0, 32, 64)  # PE column groups for e-matmul output spreading

_CACHE = {}


def _ap_bcast_t(base, nb, nt):
    """AP reading base [128, nb] broadcast over an inner t dim of size nt."""
    return bass.AP(tensor=base.tensor, offset=base.offset,
                   ap=[base.ap[0], [base.ap[-1][0], nb], [0, nt]])


def _split_excess_waits(nc, max_waits=1):
    """This container's walrus rejects instructions carrying more than
    ~max_waits semaphore waits ("Too many sync wait commands"). Hoist excess
    waits onto InstNoOp instructions inserted just before, on the same engine
    (per-engine program order makes this semantics-preserving)."""
    nid = [0]
    for f in nc.m.functions:
        for blk in f.blocks:
            insts = blk.instructions
            out = []
            changed = False
            for ins in insts:
                si = ins.sync_info
                ow = list(si.on_wait) if si is not None and si.on_wait else []
                if len(ow) > max_waits:
                    changed = True
                    while len(ow) > max_waits:
                        take, ow = ow[:max_waits], ow[max_waits:]
                        nid[0] += 1
                        nop = mybir.InstNoOp(
                            name=f"WSPLIT-{nid[0]}", engine=ins.engine,
                            sync_info=mybir.SyncInfo(on_wait=take,
                                                     on_update=[]))
                        nc.register_instruction(nop, overwrite=True)
                        out.append(nop)
                    ins.sync_info = mybir.SyncInfo(
                        on_wait=ow, on_update=list(si.on_update or []))
                out.append(ins)
            if changed:
                blk.instructions = out


def _build():
    nc = bass.Bass("TRN2", target_bir_lowering=False)
    f16, f32 = dt.float16, dt.float32

    h_nat_d = nc.declare_dram_parameter("h_nat", [BC, T, D], f16, isOutput=False)
    h_t_d = nc.declare_dram_parameter("h_t", [D, BC, T], f16, isOutput=False)
    oneh_d = nc.declare_dram_parameter("onehot", [C, S, BC], f16, isOutput=False)
    wi2h_d = nc.declare_dram_parameter("wi2h", [D, H], f16, isOutput=False)
    wh2h_d = nc.declare_dram_parameter("wh2h", [H, H], f16, isOutput=False)
    bh2h_d = nc.declare_dram_parameter("bh2hT", [H, 1], f32, isOutput=False)
    wsc_d = nc.declare_dram_parameter("wsc", [H, 32], f16, isOutput=False)
    wxc_d = nc.declare_dram_parameter("wxc", [D, 4 * H], f16, isOutput=False)
    wxo_d = nc.declare_dram_parameter("wxo", [C, 4 * H], f16, isOutput=False)
    wh_d = nc.declare_dram_parameter("wh", [H, 4 * H], f16, isOutput=False)
    wg_d = nc.declare_dram_parameter("wg", [H, C], f16, isOutput=False)
    probs_d = nc.declare_dram_parameter("probsT", [C, S, BC], f32, isOutput=True)

    with tile.TileContext(nc) as tc:
        import contextlib
        ctx = contextlib.ExitStack()
        with ctx:
            singles = ctx.enter_context(tc.tile_pool(name="singles", bufs=1))
            psA = ctx.enter_context(tc.tile_pool(name="psA", bufs=1, space="PSUM"))
            psB = ctx.enter_context(tc.tile_pool(name="psB", bufs=1, space="PSUM"))

            # ---- persistent SBUF state ----
            h_nat = singles.tile([BC, T, D], f16)
            projT = singles.tile([128, KT, BC, T], f16)  # [h', m, b, t]
            hidT = singles.tile([128, KT, S + 1, BC], f16)  # h states, slot 0 = 0
            oneh = singles.tile([C, S, BC], f16)
            wi2h = singles.tile([128, KT, H], f16)
            wh2h = singles.tile([128, KT, H], f16)
            bh2hT = singles.tile([128, KT, 1], f32)
            wsc = singles.tile([128, KT, 32], f16)
            wxc = singles.tile([128, KT, 4 * H], f16)
            wxo = singles.tile([C, 4 * H], f16)
            wh = singles.tile([128, KT, 4 * H], f16)
            wg = singles.tile([128, KT, C], f16)
            ident = singles.tile([128, 128], f16)
            cT = singles.tile([BC, H], f16)

            # ---- load everything ----
            nc.sync.dma_start(out=h_nat, in_=h_nat_d[:])
            nc.sync.dma_start(out=oneh, in_=oneh_d[:])
            nc.sync.dma_start(
                out=wi2h, in_=wi2h_d[:].rearrange("(k p) h -> p k h", p=128))
            nc.sync.dma_start(
                out=wh2h, in_=wh2h_d[:].rearrange("(k p) h -> p k h", p=128))
            nc.sync.dma_start(
                out=bh2hT, in_=bh2h_d[:].rearrange("(k p) o -> p k o", p=128))
            nc.sync.dma_start(
                out=wsc, in_=wsc_d[:].rearrange("(k p) o -> p k o", p=128))
            nc.sync.dma_start(
                out=wxc, in_=wxc_d[:].rearrange("(k p) g -> p k g", p=128))
            nc.sync.dma_start(out=wxo, in_=wxo_d[:])
            nc.sync.dma_start(
                out=wh, in_=wh_d[:].rearrange("(k p) g -> p k g", p=128))
            nc.sync.dma_start(
                out=wg, in_=wg_d[:].rearrange("(k p) c -> p k c", p=128))
            make_identity(nc, ident)
            nc.vector.memset(hidT[:, :, 0, :], 0.0)
            nc.vector.memset(cT, 0.0)

            # ---- precompute projT = (batch_H @ Wi2h)^T : [h', m, (b t)] ----
            projT_f = projT[:].rearrange("p m b t -> p m (b t)")
            with tc.tile_pool(name="ht", bufs=1) as ht_pool:
                h_tt = ht_pool.tile([128, KT, BC, T], f16)
                nc.sync.dma_start(
                    out=h_tt,
                    in_=h_t_d[:].rearrange("(k p) b t -> p k b t", p=128))
                h_tt_f = h_tt[:].rearrange("p k b t -> p k (b t)")
                for m in range(KT):
                    for c in range(TB // NCHUNK):
                        ps = psA.tile([128, 6, NCHUNK], f32, tag="e")
                        sl = slice(c * NCHUNK, (c + 1) * NCHUNK)
                        for k in range(KT):
                            nc.tensor.matmul(
                                ps[:, 0, :], wi2h[:, k, m * 128:(m + 1) * 128],
                                h_tt_f[:, k, sl], start=(k == 0),
                                stop=(k == KT - 1))
                        eng = nc.scalar if (c % 2 == 0) else nc.vector
                        if eng is nc.scalar:
                            eng.copy(out=projT_f[:, m, sl], in_=ps[:, 0, :])
                        else:
                            eng.tensor_copy(projT_f[:, m, sl], ps[:, 0, :])

            work = ctx.enter_context(tc.tile_pool(name="work", bufs=2))
            small = ctx.enter_context(tc.tile_pool(name="small", bufs=2))
            big1 = ctx.enter_context(tc.tile_pool(name="big1", bufs=1))
            ahp = ctx.enter_context(tc.tile_pool(name="ahp", bufs=16))

            # ---- decode steps ----
            for s in range(S):
                # hp = h @ Wh2h + bh2h, transposed [h', m, b]
                ps_hp = psB.tile([BC, KT, 128], f32, tag="misc")
                for m in range(KT):
                    for k in range(KT):
                        nc.tensor.matmul(
                            ps_hp[:, m, :], wh2h[:, k, m * 128:(m + 1) * 128],
                            hidT[:, k, s, :], start=(k == 0), stop=(k == KT - 1))
                hpT = small.tile([128, KT, 128], f16)
                for m in range(KT):
                    nc.vector.tensor_scalar(
                        out=hpT[:, m, :], in0=ps_hp[:, m, :],
                        scalar1=bh2hT[:, m, :], scalar2=None, op0=ALU.add)
                # replicate hp 8x in an inner dim so the z-add's broadcast AP
                # ends with a step-1 dim (keeps DVE 2x mode); per-m so the
                # z-pipeline fills sooner
                hpR = small.tile([128, KT, 128, 8], f16, tag="hpR")
                for m in range(KT):
                    base = hpT[:, m, :]
                    nc.vector.tensor_copy(
                        hpR[:, m, :, :],
                        bass.AP(tensor=base.tensor, offset=base.offset,
                                ap=[base.ap[0], [1, 128], [0, 8]]))

                # z = projT + hp (bcast over t); tanh; e = w . tanh
                ps_e = psA.tile([128, 6, NCHUNK], f32, tag="e")
                for m in range(KT):
                    cg = 0
                    b0 = 0
                    for nbz in (8, 24, 32, 64):
                        bsl = slice(b0, b0 + nbz)
                        gp = (m == KT - 1 and b0 == 64)
                        # the last chunk of m=1 runs on gpsimd (plain TT add)
                        # from a dedicated tile so it isn't starved by the
                        # z-slot rotation
                        z = work.tile([128, 64, T], f16,
                                      tag=("zgp" if gp else "z"))
                        hb = hpR[:, m, b0:, :]
                        zeng = nc.vector  # gpsimd TT crashed HW (NRT 101)
                        zeng.tensor_tensor(
                            out=z[:, :nbz, :].rearrange(
                                "p b (r i) -> p b r i", i=8),
                            in0=projT[:, m, bsl, :].rearrange(
                                "p b (r i) -> p b r i", i=8),
                            in1=bass.AP(tensor=hb.tensor, offset=hb.offset,
                                        ap=[hb.ap[0], [hb.ap[1][0], nbz],
                                            [0, T // 8], [1, 8]]),
                            op=ALU.add)
                        th = work.tile([128, 64 * T], f16, tag="th")
                        nc.scalar.activation(
                            out=th[:, :nbz * T],
                            in_=z[:, :nbz, :].rearrange("p b t -> p (b t)"),
                            func=AF.Tanh)
                        for q in range(nbz * T // NCHUNK):
                            col = 32 * (cg // 6)
                            bank = cg % 6
                            nc.tensor.matmul(
                                ps_e[col:col + 32, bank, :], wsc[:, m, :],
                                th[:, q * NCHUNK:(q + 1) * NCHUNK],
                                start=(m == 0), stop=(m == KT - 1),
                                tile_position=(0, col),
                                skip_group_check=True)
                            cg += 1
                        b0 += nbz
                    assert b0 == BC

                # softmax over t (e bounded: no max subtraction needed).
                # Chunk cg lives replicated on partitions
                # [32*(cg//6), +32), bank cg%6. Valid rectangles:
                # [0:96, banks 0:4] and [0:64, banks 4:6].
                expa = big1.tile([128, 6, NCHUNK], f32, tag="expa")
                nc.scalar.activation(out=expa[0:64, 4:6, :],
                                     in_=ps_e[0:64, 4:6, :], func=AF.Exp)
                nc.scalar.activation(out=expa[0:96, 0:4, :],
                                     in_=ps_e[0:96, 0:4, :], func=AF.Exp)
                # unnormalized alpha scatter -> [128 b, 64 t]; chunk cg is
                # on partition-row 32*(cg//6), bank cg%6, so rows 0/32 cover
                # chunks 0..11 (-> b 0..96) and row 64 banks 0..3 -> b 96..128
                alpha = small.tile([BC, T], f32, tag="alpha")
                for row, nb in ((0, 6), (1, 6), (2, 4)):
                    er = expa[32 * row:32 * row + 1, 0:nb, :]
                    nc.sync.dma_start(
                        out=alpha[48 * row:48 * row + 8 * nb, :],
                        in_=bass.AP(tensor=er.tensor, offset=er.offset,
                                    ap=[er.ap[0], [NCHUNK, nb], [1, NCHUNK]]))
                # Z = sum_t exp in [b, t] layout; rzb = 1/Z  [128, 1]
                zsum = small.tile([BC, 1], f32, tag="zsum")
                nc.vector.reduce_sum(out=zsum, in_=alpha,
                                     axis=mybir.AxisListType.X)
                rzb = small.tile([BC, 1], f32, tag="rzb")
                nc.vector.reciprocal(out=rzb, in_=zsum)
                if s == 0:
                    _CACHE.setdefault("dbg", {})["alpha0"] = alpha.tensor.name
                    _CACHE["dbg"]["expa0"] = expa.tensor.name
                    _CACHE["dbg"]["hpT0"] = hpT.tensor.name
                # context = sum_t alpha~[:, t] * H[:, t, :] (unnormalized):
                # DVE makes alpha_t*H_t slices at 4x, PE accumulates them into
                # PSUM (fp32) via identity matmuls; the final copy rescales by
                # rzb = 1/Z.
                ps_cx = psB.tile([BC, D], f32, tag="misc")
                ahs = []
                for t in range(T):
                    ah = ahp.tile([BC, D], f16, tag="ah")
                    nc.vector.tensor_scalar(
                        out=ah, in0=h_nat[:, t, :], scalar1=alpha[:, t:t + 1],
                        scalar2=None, op0=ALU.mult)
                    ahs.append(ah)
                    if t % 16 == 15:
                        for j, a in enumerate(ahs):
                            tt = t - len(ahs) + 1 + j
                            nc.tensor.matmul(ps_cx, ident, a, start=(tt == 0),
                                             stop=(tt == T - 1))
                        ahs = []
                # gates partial (h and onehot contributions don't need ctx)
                ps_g = psA.tile([BC, 4 * H], f32, tag="e")
                for nchk in range(2):
                    nsl = slice(nchk * NCHUNK, (nchk + 1) * NCHUNK)
                    for k in range(KT):
                        nc.tensor.matmul(
                            ps_g[:, nsl], hidT[:, k, s, :], wh[:, k, nsl],
                            start=(k == 0), stop=False)
                    nc.tensor.matmul(
                        ps_g[:, nsl], oneh[:, s, :], wxo[:, nsl],
                        start=False, stop=False)
                ctxv = small.tile([BC, D], f16, tag="ctxv")
                nc.vector.tensor_scalar(
                    out=ctxv, in0=ps_cx, scalar1=rzb[:, 0:1], scalar2=None,
                    op0=ALU.mult)
                if s == 0:
                    _CACHE["dbg"]["ctx0"] = ctxv.tensor.name
                    _CACHE["dbg"]["projT"] = projT.tensor.name
                    _CACHE["dbg"]["hidT"] = hidT.tensor.name

                # transpose context -> [d', m, b]
                ps_ct = psB.tile([BC, KT, 128], f16, tag="misc")
                ctxT = small.tile([128, KT, 128], f16, tag="ctxT")
                for m in range(KT):
                    nc.tensor.transpose(
                        ps_ct[:, m, :], ctxv[:, m * 128:(m + 1) * 128], ident)
                    nc.vector.tensor_copy(ctxT[:, m, :], ps_ct[:, m, :])

                if s == 0:
                    _CACHE["dbg"]["ctxT0"] = ctxT.tensor.name
                # gates[b, 4H] = h.Wh + onehot.Wxo + ctx.Wxc with the
                # activations as the stationary operand (5 weight loads, 10
                # N=512 matmuls) instead of 40 small ones.
                for nchk in range(2):
                    nsl = slice(nchk * NCHUNK, (nchk + 1) * NCHUNK)
                    for k in range(KT):
                        nc.tensor.matmul(
                            ps_g[:, nsl], ctxT[:, k, :], wxc[:, k, nsl],
                            start=False, stop=(k == KT - 1))

                # LSTM pointwise in [b, g] layout. keras order i, f, g, o
                # (b_lstm folded into wxo host-side).
                sig_if = small.tile([BC, 2 * H], f16, tag="sig_if")
                tg = small.tile([BC, H], f16, tag="tg")
                sig_o = small.tile([BC, H], f16, tag="sig_o")
                nc.scalar.activation(out=sig_if, in_=ps_g[:, 0:2 * H],
                                     func=AF.Sigmoid)
                nc.scalar.activation(out=tg, in_=ps_g[:, 2 * H:3 * H],
                                     func=AF.Tanh)
                nc.scalar.activation(out=sig_o, in_=ps_g[:, 3 * H:4 * H],
                                     func=AF.Sigmoid)
                t1 = small.tile([BC, H], f16, tag="t1")
                t2 = small.tile([BC, H], f16, tag="t2")
                nc.vector.tensor_tensor(out=t1, in0=sig_if[:, H:2 * H],
                                        in1=cT, op=ALU.mult)
                nc.vector.tensor_tensor(out=t2, in0=sig_if[:, 0:H], in1=tg,
                                        op=ALU.mult)
                nc.vector.tensor_tensor(out=cT, in0=t1, in1=t2, op=ALU.add)
                tc_t = small.tile([BC, H], f16, tag="tc_t")
                nc.scalar.activation(out=tc_t, in_=cT, func=AF.Tanh)
                h_bd = small.tile([BC, H], f16, tag="h_bd")
                nc.vector.tensor_tensor(out=h_bd, in0=sig_o, in1=tc_t,
                                        op=ALU.mult)
                # transpose h back to [h', k, b] for the next step's matmuls
                ps_h = psB.tile([BC, KT, 128], f16, tag="misc")
                for k in range(KT):
                    nc.tensor.transpose(
                        ps_h[:, k, :], h_bd[:, k * 128:(k + 1) * 128], ident)
                    nc.vector.tensor_copy(hidT[:, k, s + 1, :], ps_h[:, k, :])

            # ---- generator: probsT = Wg^T . h_s for all steps ----
            hid_f = hidT[:].rearrange("p k s b -> p k (s b)")
            NS = S * BC  # 3328
            probs_sb = singles.tile([C, S, BC], f32)
            probs_f = probs_sb[:].rearrange("c s b -> c (s b)")
            pos = 0
            ci = 0
            while pos < NS:
                n = min(NCHUNK, NS - pos)
                ps_p = psB.tile([C, NCHUNK], f32, tag="misc")
                for k in range(KT):
                    nc.tensor.matmul(
                        ps_p[:, :n], wg[:, k, :], hid_f[:, k, BC + pos:BC + pos + n],
                        start=(k == 0), stop=(k == KT - 1))
                eng = nc.scalar if (ci % 2 == 0) else nc.vector
                if eng is nc.scalar:
                    eng.copy(out=probs_f[:, pos:pos + n], in_=ps_p[:, :n])
                else:
                    eng.tensor_copy(probs_f[:, pos:pos + n], ps_p[:, :n])
                pos += n
                ci += 1
            nc.sync.dma_start(out=probs_d[:], in_=probs_sb)

    _split_excess_waits(nc)
    return nc


def _get_module():
    if "nc" not in _CACHE:
        _CACHE["nc"] = _build()
    return _CACHE["nc"]


def build_in_maps(batch_H, text, batch_max_length, Wi2h, Wh2h, bh2h, w_score,
                  Wx, Wh, b_lstm, Wg, bg):
    batch_H = np.asarray(batch_H, dtype=np.float32)
    text = np.asarray(text)
    assert int(batch_max_length) + 1 == S
    assert batch_H.shape == (B, T, D)

    f16 = np.float16
    bh16 = batch_H.astype(f16)
    # one-hot text: [B, S, C] -> per-core [S, C, BC]
    oh = (text[:, :S, None] == np.arange(C)[None, None, :])

    Wx = np.asarray(Wx, np.float32)
    wxo_p = (Wx[D:D + C, :] + np.asarray(b_lstm, np.float32)[None, :]).astype(f16)
    weights = {
        "wi2h": np.ascontiguousarray(np.asarray(Wi2h, np.float32).astype(f16)),
        "wh2h": np.ascontiguousarray(np.asarray(Wh2h, np.float32).astype(f16)),
        "bh2hT": np.ascontiguousarray(
            np.asarray(bh2h, np.float32).reshape(H, 1)),
        "wsc": np.ascontiguousarray(np.tile(
            np.asarray(w_score, np.float32).reshape(H, 1), (1, 32)).astype(f16)),
        "wxc": np.ascontiguousarray(Wx[:D, :].astype(f16)),
        "wxo": np.ascontiguousarray(wxo_p),
        "wh": np.ascontiguousarray(np.asarray(Wh, np.float32).astype(f16)),
        "wg": np.ascontiguousarray(np.asarray(Wg, np.float32).astype(f16)),
    }

    in_maps = []
    for c in range(NCORES):
        bsl = slice(c * BC, (c + 1) * BC)
        in_maps.append({
            "h_nat": np.ascontiguousarray(bh16[bsl]),
            "h_t": np.ascontiguousarray(bh16[bsl].transpose(2, 0, 1)),
            "onehot": np.ascontiguousarray(
                oh[bsl].transpose(2, 1, 0).astype(f16)),
            **weights,
        })
    return in_maps


def kernel(**inputs):
    in_maps = build_in_maps(**inputs)
    bg = inputs["bg"]

    nc = _get_module()
    res = run_bass_kernel_spmd(nc, in_maps, list(range(NCORES)))

    out = np.empty((B, S, C), np.float32)
    for c in range(NCORES):
        out[c * BC:(c + 1) * BC] = res.results[c]["probsT"].transpose(2, 1, 0)
    out += np.asarray(bg, np.float32)[None, None, :]
    return out


if __name__ == "__main__":
    _build()
    print("build OK")


# revision 40
# speedup vs baseline: 1.5313x; 1.2254x over previous
"""Trainium2 Bass kernel for an attention-LSTM decoder (Bahdanau attention +
LSTM cell + generator head), data-parallel over 8 NeuronCores.

Shapes (hardcoded): B=1024, T=64, D=256, H=256, C=96, steps S=26.
Each core handles 128 batch rows.

Device layouts (per core, b = 128):
  - score chain runs "transposed": partitions = hidden dim tiles (2 x 128),
    free = (b, t) flat b-major.
  - softmax/context run natural: partitions = b, free = t / d.
  - LSTM/gates run transposed: gatesT [4H -> 8 tiles of 128, b].
Matmul operands are fp16 (full-rate PE streaming, 10-bit mantissa); PSUM
accumulation is fp32; the recurrent c state is fp32.

Host-side prep (numpy): fp16 casts, batch_H transpose for the projection
matmul, one-hot text encoding, b_lstm folded into the one-hot weight rows
(valid because one-hot rows sum to 1), bg added to the final output.
"""

import sys

for _p in ("/opt/trn_rl_repo", "/root/.axon_site/_ro/trn_rl_repo"):
    if _p not in sys.path:
        sys.path.insert(0, _p)

import numpy as np

import concourse.bass as bass
import concourse.tile as tile
from concourse import mybir
from concourse.bass_utils import run_bass_kernel_spmd
from concourse.masks import make_identity

dt = mybir.dt
AF = mybir.ActivationFunctionType
ALU = mybir.AluOpType

NCORES = 8
B, T, D, H, C = 1024, 64, 256, 256, 96
S = 26  # num steps = batch_max_length + 1
BC = B // NCORES  # 128 batch rows per core
KT = 2  # 256 = 2 x 128 tiles for d/h contraction
GT = 8  # 4H = 1024 = 8 m-tiles of 128
TB = BC * T  # 8192, flat (b, t) b-major
NCHUNK = 512  # psum-bank-limited matmul N
ZCH = 4096  # z/tanh chunk along flat (b, t): 64 b x 64 t
COLG = (0, 32, 64)  # PE column groups for e-matmul output spreading

_CACHE = {}


def _ap_bcast_t(base, nb, nt):
    """AP reading base [128, nb] broadcast over an inner t dim of size nt."""
    return bass.AP(tensor=base.tensor, offset=base.offset,
                   ap=[base.ap[0], [base.ap[-1][0], nb], [0, nt]])


def _split_excess_waits(nc, max_waits=1):
    """This container's walrus rejects instructions carrying more than
    ~max_waits semaphore waits ("Too many sync wait commands"). Hoist excess
    waits onto InstNoOp instructions inserted just before, on the same engine
    (per-engine program order makes this semantics-preserving)."""
    nid = [0]
    for f in nc.m.functions:
        for blk in f.blocks:
            insts = blk.instructions
            out = []
            changed = False
            for ins in insts:
                si = ins.sync_info
                ow = list(si.on_wait) if si is not None and si.on_wait else []
                if len(ow) > max_waits:
                    changed = True
                    while len(ow) > max_waits:
                        take, ow = ow[:max_waits], ow[max_waits:]
                        nid[0] += 1
                        nop = mybir.InstNoOp(
                            name=f"WSPLIT-{nid[0]}", engine=ins.engine,
                            sync_info=mybir.SyncInfo(on_wait=take,
                                                     on_update=[]))
                        nc.register_instruction(nop, overwrite=True)
                        out.append(nop)
                    ins.sync_info = mybir.SyncInfo(
                        on_wait=ow, on_update=list(si.on_update or []))
                out.append(ins)
            if changed:
                blk.instructions = out


def _build():
    nc = bass.Bass("TRN2", target_bir_lowering=False)
    f16, f32 = dt.float16, dt.float32

    h_nat_d = nc.declare_dram_parameter("h_nat", [BC, T, D], f16, isOutput=False)
    h_t_d = nc.declare_dram_parameter("h_t", [D, BC, T], f16, isOutput=False)
    oneh_d = nc.declare_dram_parameter("onehot", [C, S, BC], f16, isOutput=False)
    wi2h_d = nc.declare_dram_parameter("wi2h", [D, H], f16, isOutput=False)
    wh2h_d = nc.declare_dram_parameter("wh2h", [H, H], f16, isOutput=False)
    bh2h_d = nc.declare_dram_parameter("bh2hT", [H, 1], f32, isOutput=False)
    wsc_d = nc.declare_dram_parameter("wsc", [H, 32], f16, isOutput=False)
    wxc_d = nc.declare_dram_parameter("wxc", [D, 4 * H], f16, isOutput=False)
    wxo_d = nc.declare_dram_parameter("wxo", [C, 4 * H], f16, isOutput=False)
    wh_d = nc.declare_dram_parameter("wh", [H, 4 * H], f16, isOutput=False)
    wg_d = nc.declare_dram_parameter("wg", [H, C], f16, isOutput=False)
    probs_d = nc.declare_dram_parameter("probsT", [C, S, BC], f32, isOutput=True)

    with tile.TileContext(nc) as tc:
        import contextlib
        ctx = contextlib.ExitStack()
        with ctx:
            singles = ctx.enter_context(tc.tile_pool(name="singles", bufs=1))
            psA = ctx.enter_context(tc.tile_pool(name="psA", bufs=1, space="PSUM"))
            psB = ctx.enter_context(tc.tile_pool(name="psB", bufs=1, space="PSUM"))

            # ---- persistent SBUF state ----
            h_nat = singles.tile([BC, T, D], f16)
            projT = singles.tile([128, KT, BC, T], f16)  # [h', m, b, t]
            hidT = singles.tile([128, KT, S + 1, BC], f16)  # h states, slot 0 = 0
            oneh = singles.tile([C, S, BC], f16)
            wi2h = singles.tile([128, KT, H], f16)
            wh2h = singles.tile([128, KT, H], f16)
            bh2hT = singles.tile([128, KT, 1], f32)
            wsc = singles.tile([128, KT, 32], f16)
            wxc = singles.tile([128, KT, 4 * H], f16)
            wxo = singles.tile([C, 4 * H], f16)
            wh = singles.tile([128, KT, 4 * H], f16)
            wg = singles.tile([128, KT, C], f16)
            ident = singles.tile([128, 128], f16)
            cT = singles.tile([BC, H], f16)

            # ---- load everything ----
            nc.sync.dma_start(out=h_nat, in_=h_nat_d[:])
            nc.sync.dma_start(out=oneh, in_=oneh_d[:])
            nc.sync.dma_start(
                out=wi2h, in_=wi2h_d[:].rearrange("(k p) h -> p k h", p=128))
            nc.sync.dma_start(
                out=wh2h, in_=wh2h_d[:].rearrange("(k p) h -> p k h", p=128))
            nc.sync.dma_start(
                out=bh2hT, in_=bh2h_d[:].rearrange("(k p) o -> p k o", p=128))
            nc.sync.dma_start(
                out=wsc, in_=wsc_d[:].rearrange("(k p) o -> p k o", p=128))
            nc.sync.dma_start(
                out=wxc, in_=wxc_d[:].rearrange("(k p) g -> p k g", p=128))
            nc.sync.dma_start(out=wxo, in_=wxo_d[:])
            nc.sync.dma_start(
                out=wh, in_=wh_d[:].rearrange("(k p) g -> p k g", p=128))
            nc.sync.dma_start(
                out=wg, in_=wg_d[:].rearrange("(k p) c -> p k c", p=128))
            make_identity(nc, ident)
            nc.vector.memset(hidT[:, :, 0, :], 0.0)
            nc.vector.memset(cT, 0.0)

            # ---- precompute projT = (batch_H @ Wi2h)^T : [h', m, (b t)] ----
            projT_f = projT[:].rearrange("p m b t -> p m (b t)")
            with tc.tile_pool(name="ht", bufs=1) as ht_pool:
                h_tt = ht_pool.tile([128, KT, BC, T], f16)
                nc.sync.dma_start(
                    out=h_tt,
                    in_=h_t_d[:].rearrange("(k p) b t -> p k b t", p=128))
                h_tt_f = h_tt[:].rearrange("p k b t -> p k (b t)")
                for m in range(KT):
                    for c in range(TB // NCHUNK):
                        ps = psA.tile([128, 4, NCHUNK], f32, tag="e")
                        sl = slice(c * NCHUNK, (c + 1) * NCHUNK)
                        for k in range(KT):
                            nc.tensor.matmul(
                                ps[:, 0, :], wi2h[:, k, m * 128:(m + 1) * 128],
                                h_tt_f[:, k, sl], start=(k == 0),
                                stop=(k == KT - 1))
                        eng = nc.scalar if (c % 2 == 0) else nc.vector
                        if eng is nc.scalar:
                            eng.copy(out=projT_f[:, m, sl], in_=ps[:, 0, :])
                        else:
                            eng.tensor_copy(projT_f[:, m, sl], ps[:, 0, :])

            work = ctx.enter_context(tc.tile_pool(name="work", bufs=2))
            small = ctx.enter_context(tc.tile_pool(name="small", bufs=2))
            big1 = ctx.enter_context(tc.tile_pool(name="big1", bufs=1))
            ahp = ctx.enter_context(tc.tile_pool(name="ahp", bufs=16))

            # ---- decode steps ----
            for s in range(S):
                # hp = h @ Wh2h + bh2h, transposed [h', m, b]
                ps_hp = psB.tile([BC, KT, 128], f32, tag="misc")
                for m in range(KT):
                    for k in range(KT):
                        nc.tensor.matmul(
                            ps_hp[:, m, :], wh2h[:, k, m * 128:(m + 1) * 128],
                            hidT[:, k, s, :], start=(k == 0), stop=(k == KT - 1))
                hpT = small.tile([128, KT, 128], f16)
                for m in range(KT):
                    nc.vector.tensor_scalar(
                        out=hpT[:, m, :], in0=ps_hp[:, m, :],
                        scalar1=bh2hT[:, m, :], scalar2=None, op0=ALU.add)
                # replicate hp 8x in an inner dim so the z-add's broadcast AP
                # ends with a step-1 dim (keeps DVE 2x mode); per-m so the
                # z-pipeline fills sooner
                hpR = small.tile([128, KT, 128, 8], f16, tag="hpR")
                for m in range(KT):
                    base = hpT[:, m, :]
                    nc.vector.tensor_copy(
                        hpR[:, m, :, :],
                        bass.AP(tensor=base.tensor, offset=base.offset,
                                ap=[base.ap[0], [1, 128], [0, 8]]))

                # z = projT + hp (bcast over t); tanh; e = w . tanh
                ps_e = psA.tile([128, 4, NCHUNK], f32, tag="e")
                for m in range(KT):
                    cg = 0
                    b0 = 0
                    for nbz in (8, 24, 32, 64):
                        bsl = slice(b0, b0 + nbz)
                        gp = (m == KT - 1 and b0 == 64)
                        # the last chunk of m=1 runs on gpsimd (plain TT add)
                        # from a dedicated tile so it isn't starved by the
                        # z-slot rotation
                        z = work.tile([128, 64, T], f16,
                                      tag=("zgp" if gp else "z"))
                        hb = hpR[:, m, b0:, :]
                        zeng = nc.vector  # gpsimd TT crashed HW (NRT 101)
                        zeng.tensor_tensor(
                            out=z[:, :nbz, :].rearrange(
                                "p b (r i) -> p b r i", i=8),
                            in0=projT[:, m, bsl, :].rearrange(
                                "p b (r i) -> p b r i", i=8),
                            in1=bass.AP(tensor=hb.tensor, offset=hb.offset,
                                        ap=[hb.ap[0], [hb.ap[1][0], nbz],
                                            [0, T // 8], [1, 8]]),
                            op=ALU.add)
                        th = work.tile([128, 64 * T], f16, tag="th")
                        nc.scalar.activation(
                            out=th[:, :nbz * T],
                            in_=z[:, :nbz, :].rearrange("p b t -> p (b t)"),
                            func=AF.Tanh)
                        for q in range(nbz * T // NCHUNK):
                            col = 32 * (cg // 4)
                            bank = cg % 4
                            nc.tensor.matmul(
                                ps_e[col:col + 32, bank, :], wsc[:, m, :],
                                th[:, q * NCHUNK:(q + 1) * NCHUNK],
                                start=(m == 0), stop=(m == KT - 1),
                                tile_position=(0, col),
                                skip_group_check=True)
                            cg += 1
                        b0 += nbz
                    assert b0 == BC

                # softmax over t (e bounded: no max subtraction needed).
                # Chunk cg lives replicated on partitions
                # [32*(cg//6), +32), bank cg%6. Valid rectangles:
                # [0:96, banks 0:4] and [0:64, banks 4:6].
                expa = big1.tile([128, 4, NCHUNK], f32, tag="expa")
                nc.scalar.activation(out=expa[0:128, 0:4, :],
                                     in_=ps_e[0:128, 0:4, :], func=AF.Exp)
                # unnormalized alpha scatter -> [128 b, 64 t]; chunk cg is
                # on partition-row 32*(cg//6), bank cg%6, so rows 0/32 cover
                # chunks 0..11 (-> b 0..96) and row 64 banks 0..3 -> b 96..128
                alpha = small.tile([BC, T], f32, tag="alpha")
                qeng = (nc.sync, nc.scalar, nc.sync, nc.scalar)
                for row in range(4):
                    er = expa[32 * row:32 * row + 1, 0:4, :]
                    qeng[row].dma_start(
                        out=alpha[32 * row:32 * row + 32, :],
                        in_=bass.AP(tensor=er.tensor, offset=er.offset,
                                    ap=[er.ap[0], [NCHUNK, 4], [1, NCHUNK]]))
                # Z = sum_t exp in [b, t] layout; rzb = 1/Z  [128, 1]
                zsum = small.tile([BC, 1], f32, tag="zsum")
                nc.vector.reduce_sum(out=zsum, in_=alpha,
                                     axis=mybir.AxisListType.X)
                rzb = small.tile([BC, 1], f32, tag="rzb")
                nc.vector.reciprocal(out=rzb, in_=zsum)
                if s == 0:
                    _CACHE.setdefault("dbg", {})["alpha0"] = alpha.tensor.name
                    _CACHE["dbg"]["expa0"] = expa.tensor.name
                    _CACHE["dbg"]["hpT0"] = hpT.tensor.name
                # context = sum_t alpha~[:, t] * H[:, t, :] (unnormalized):
                # DVE makes alpha_t*H_t slices at 4x, PE accumulates them into
                # PSUM (fp32) via identity matmuls; the final copy rescales by
                # rzb = 1/Z.
                ps_cx = psB.tile([BC, D], f32, tag="misc")
                ahs = []
                for t in range(T):
                    ah = ahp.tile([BC, D], f16, tag="ah")
                    nc.vector.tensor_scalar(
                        out=ah, in0=h_nat[:, t, :], scalar1=alpha[:, t:t + 1],
                        scalar2=None, op0=ALU.mult)
                    ahs.append(ah)
                    if t % 16 == 15:
                        for j, a in enumerate(ahs):
                            tt = t - len(ahs) + 1 + j
                            nc.tensor.matmul(ps_cx, ident, a, start=(tt == 0),
                                             stop=(tt == T - 1))
                        ahs = []
                # gates partial (h and onehot contributions don't need ctx)
                ps_g = psA.tile([BC, 4 * H], f32, tag="e")
                for nchk in range(2):
                    nsl = slice(nchk * NCHUNK, (nchk + 1) * NCHUNK)
                    for k in range(KT):
                        nc.tensor.matmul(
                            ps_g[:, nsl], hidT[:, k, s, :], wh[:, k, nsl],
                            start=(k == 0), stop=False)
                    nc.tensor.matmul(
                        ps_g[:, nsl], oneh[:, s, :], wxo[:, nsl],
                        start=False, stop=False)
                ctxv = small.tile([BC, D], f16, tag="ctxv")
                nc.vector.tensor_scalar(
                    out=ctxv, in0=ps_cx, scalar1=rzb[:, 0:1], scalar2=None,
                    op0=ALU.mult)
                if s == 0:
                    _CACHE["dbg"]["ctx0"] = ctxv.tensor.name
                    _CACHE["dbg"]["projT"] = projT.tensor.name
                    _CACHE["dbg"]["hidT"] = hidT.tensor.name

                # transpose context -> [d', m, b]
                ps_ct = psB.tile([BC, KT, 128], f16, tag="misc")
                ctxT = small.tile([128, KT, 128], f16, tag="ctxT")
                for m in range(KT):
                    nc.tensor.transpose(
                        ps_ct[:, m, :], ctxv[:, m * 128:(m + 1) * 128], ident)
                    nc.vector.tensor_copy(ctxT[:, m, :], ps_ct[:, m, :])

                if s == 0:
                    _CACHE["dbg"]["ctxT0"] = ctxT.tensor.name
                # gates[b, 4H] = h.Wh + onehot.Wxo + ctx.Wxc with the
                # activations as the stationary operand (5 weight loads, 10
                # N=512 matmuls) instead of 40 small ones.
                for nchk in range(2):
                    nsl = slice(nchk * NCHUNK, (nchk + 1) * NCHUNK)
                    for k in range(KT):
                        nc.tensor.matmul(
                            ps_g[:, nsl], ctxT[:, k, :], wxc[:, k, nsl],
                            start=False, stop=(k == KT - 1))

                # LSTM pointwise in [b, g] layout. keras order i, f, g, o
                # (b_lstm folded into wxo host-side).
                sig_if = small.tile([BC, 2 * H], f16, tag="sig_if")
                tg = small.tile([BC, H], f16, tag="tg")
                sig_o = small.tile([BC, H], f16, tag="sig_o")
                nc.scalar.activation(out=sig_if, in_=ps_g[:, 0:2 * H],
                                     func=AF.Sigmoid)
                nc.scalar.activation(out=tg, in_=ps_g[:, 2 * H:3 * H],
                                     func=AF.Tanh)
                nc.scalar.activation(out=sig_o, in_=ps_g[:, 3 * H:4 * H],
                                     func=AF.Sigmoid)
                t1 = small.tile([BC, H], f16, tag="t1")
                t2 = small.tile([BC, H], f16, tag="t2")
                nc.vector.tensor_tensor(out=t1, in0=sig_if[:, H:2 * H],
                                        in1=cT, op=ALU.mult)
                nc.vector.tensor_tensor(out=t2, in0=sig_if[:, 0:H], in1=tg,
                                        op=ALU.mult)
                nc.vector.tensor_tensor(out=cT, in0=t1, in1=t2, op=ALU.add)
                tc_t = small.tile([BC, H], f16, tag="tc_t")
                nc.scalar.activation(out=tc_t, in_=cT, func=AF.Tanh)
                h_bd = small.tile([BC, H], f16, tag="h_bd")
                nc.vector.tensor_tensor(out=h_bd, in0=sig_o, in1=tc_t,
                                        op=ALU.mult)
                # transpose h back to [h', k, b] for the next step's matmuls
                ps_h = psB.tile([BC, KT, 128], f16, tag="misc")
                for k in range(KT):
                    nc.tensor.transpose(
                        ps_h[:, k, :], h_bd[:, k * 128:(k + 1) * 128], ident)
                    nc.vector.tensor_copy(hidT[:, k, s + 1, :], ps_h[:, k, :])

            # ---- generator: probsT = Wg^T . h_s for all steps ----
            hid_f = hidT[:].rearrange("p k s b -> p k (s b)")
            NS = S * BC  # 3328
            probs_sb = singles.tile([C, S, BC], f32)
            probs_f = probs_sb[:].rearrange("c s b -> c (s b)")
            pos = 0
            ci = 0
            while pos < NS:
                n = min(NCHUNK, NS - pos)
                ps_p = psB.tile([C, NCHUNK], f32, tag="misc")
                for k in range(KT):
                    nc.tensor.matmul(
                        ps_p[:, :n], wg[:, k, :], hid_f[:, k, BC + pos:BC + pos + n],
                        start=(k == 0), stop=(k == KT - 1))
                eng = nc.scalar if (ci % 2 == 0) else nc.vector
                if eng is nc.scalar:
                    eng.copy(out=probs_f[:, pos:pos + n], in_=ps_p[:, :n])
                else:
                    eng.tensor_copy(probs_f[:, pos:pos + n], ps_p[:, :n])
                pos += n
                ci += 1
            nc.sync.dma_start(out=probs_d[:], in_=probs_sb)

    _split_excess_waits(nc)
    return nc


def _get_module():
    if "nc" not in _CACHE:
        _CACHE["nc"] = _build()
    return _CACHE["nc"]


def build_in_maps(batch_H, text, batch_max_length, Wi2h, Wh2h, bh2h, w_score,
                  Wx, Wh, b_lstm, Wg, bg):
    batch_H = np.asarray(batch_H, dtype=np.float32)
    text = np.asarray(text)
    assert int(batch_max_length) + 1 == S
    assert batch_H.shape == (B, T, D)

    f16 = np.float16
    bh16 = batch_H.astype(f16)
    # one-hot text: [B, S, C] -> per-core [S, C, BC]
    oh = (text[:, :S, None] == np.arange(C)[None, None, :])

    Wx = np.asarray(Wx, np.float32)
    wxo_p = (Wx[D:D + C, :] + np.asarray(b_lstm, np.float32)[None, :]).astype(f16)
    weights = {
        "wi2h": np.ascontiguousarray(np.asarray(Wi2h, np.float32).astype(f16)),
        "wh2h": np.ascontiguousarray(np.asarray(Wh2h, np.float32).astype(f16)),
        "bh2hT": np.ascontiguousarray(
            np.asarray(bh2h, np.float32).reshape(H, 1)),
        "wsc": np.ascontiguousarray(np.tile(
            np.asarray(w_score, np.float32).reshape(H, 1), (1, 32)).astype(f16)),
        "wxc": np.ascontiguousarray(Wx[:D, :].astype(f16)),
        "wxo": np.ascontiguousarray(wxo_p),
        "wh": np.ascontiguousarray(np.asarray(Wh, np.float32).astype(f16)),
        "wg": np.ascontiguousarray(np.asarray(Wg, np.float32).astype(f16)),
    }

    in_maps = []
    for c in range(NCORES):
        bsl = slice(c * BC, (c + 1) * BC)
        in_maps.append({
            "h_nat": np.ascontiguousarray(bh16[bsl]),
            "h_t": np.ascontiguousarray(bh16[bsl].transpose(2, 0, 1)),
            "onehot": np.ascontiguousarray(
                oh[bsl].transpose(2, 1, 0).astype(f16)),
            **weights,
        })
    return in_maps


def kernel(**inputs):
    in_maps = build_in_maps(**inputs)
    bg = inputs["bg"]

    nc = _get_module()
    res = run_bass_kernel_spmd(nc, in_maps, list(range(NCORES)))

    out = np.empty((B, S, C), np.float32)
    for c in range(NCORES):
        out[c * BC:(c + 1) * BC] = res.results[c]["probsT"].transpose(2, 1, 0)
    out += np.asarray(bg, np.float32)[None, None, :]
    return out


if __name__ == "__main__":
    _build()
    print("build OK")


# revision 49
# speedup vs baseline: 1.7187x; 1.1224x over previous
"""Trainium2 Bass kernel for an attention-LSTM decoder (Bahdanau attention +
LSTM cell + generator head), data-parallel over 8 NeuronCores.

Shapes (hardcoded): B=1024, T=64, D=256, H=256, C=96, steps S=26.
Each core handles 128 batch rows.

Device layouts (per core, b = 128):
  - score chain runs "transposed": partitions = hidden dim tiles (2 x 128),
    free = (b, t) flat b-major.
  - softmax/context run natural: partitions = b, free = t / d.
  - LSTM/gates run transposed: gatesT [4H -> 8 tiles of 128, b].
Matmul operands are fp16 (full-rate PE streaming, 10-bit mantissa); PSUM
accumulation is fp32; the recurrent c state is fp32.

Host-side prep (numpy): fp16 casts, batch_H transpose for the projection
matmul, one-hot text encoding, b_lstm folded into the one-hot weight rows
(valid because one-hot rows sum to 1), bg added to the final output.
"""

import sys

for _p in ("/opt/trn_rl_repo", "/root/.axon_site/_ro/trn_rl_repo"):
    if _p not in sys.path:
        sys.path.insert(0, _p)

import numpy as np

import concourse.bass as bass
import concourse.tile as tile
from concourse import mybir
from concourse.bass_utils import run_bass_kernel_spmd
from concourse.masks import make_identity

dt = mybir.dt
AF = mybir.ActivationFunctionType
ALU = mybir.AluOpType

NCORES = 8
B, T, D, H, C = 1024, 64, 256, 256, 96
S = 26  # num steps = batch_max_length + 1
BC = B // NCORES  # 128 batch rows per core
KT = 2  # 256 = 2 x 128 tiles for d/h contraction
GT = 8  # 4H = 1024 = 8 m-tiles of 128
TB = BC * T  # 8192, flat (b, t) b-major
NCHUNK = 512  # psum-bank-limited matmul N
ZCH = 4096  # z/tanh chunk along flat (b, t): 64 b x 64 t
COLG = (0, 32, 64)  # PE column groups for e-matmul output spreading

_CACHE = {}


def _ap_bcast_t(base, nb, nt):
    """AP reading base [128, nb] broadcast over an inner t dim of size nt."""
    return bass.AP(tensor=base.tensor, offset=base.offset,
                   ap=[base.ap[0], [base.ap[-1][0], nb], [0, nt]])


def _split_excess_waits(nc, max_waits=1):
    """This container's walrus rejects instructions carrying more than
    ~max_waits semaphore waits ("Too many sync wait commands"). Hoist excess
    waits onto InstNoOp instructions inserted just before, on the same engine
    (per-engine program order makes this semantics-preserving)."""
    nid = [0]
    for f in nc.m.functions:
        for blk in f.blocks:
            insts = blk.instructions
            out = []
            changed = False
            for ins in insts:
                si = ins.sync_info
                ow = list(si.on_wait) if si is not None and si.on_wait else []
                if len(ow) > max_waits:
                    changed = True
                    while len(ow) > max_waits:
                        take, ow = ow[:max_waits], ow[max_waits:]
                        nid[0] += 1
                        nop = mybir.InstNoOp(
                            name=f"WSPLIT-{nid[0]}", engine=ins.engine,
                            sync_info=mybir.SyncInfo(on_wait=take,
                                                     on_update=[]))
                        nc.register_instruction(nop, overwrite=True)
                        out.append(nop)
                    ins.sync_info = mybir.SyncInfo(
                        on_wait=ow, on_update=list(si.on_update or []))
                out.append(ins)
            if changed:
                blk.instructions = out


def _build():
    nc = bass.Bass("TRN2", target_bir_lowering=False)
    f16, f32 = dt.float16, dt.float32

    h_nat_d = nc.declare_dram_parameter("h_nat", [BC, T, D], f16, isOutput=False)
    h_t_d = nc.declare_dram_parameter("h_t", [D, BC, T], f16, isOutput=False)
    oneh_d = nc.declare_dram_parameter("onehot", [C, S, BC], f16, isOutput=False)
    wi2h_d = nc.declare_dram_parameter("wi2h", [D, H], f16, isOutput=False)
    wh2h_d = nc.declare_dram_parameter("wh2h", [H, H], f16, isOutput=False)
    bh2h_d = nc.declare_dram_parameter("bh2hT", [H, 1], f32, isOutput=False)
    wsc_d = nc.declare_dram_parameter("wsc", [H, 32], f16, isOutput=False)
    wxc_d = nc.declare_dram_parameter("wxc", [D, 4 * H], f16, isOutput=False)
    wxo_d = nc.declare_dram_parameter("wxo", [C, 4 * H], f16, isOutput=False)
    wh_d = nc.declare_dram_parameter("wh", [H, 4 * H], f16, isOutput=False)
    wg_d = nc.declare_dram_parameter("wg", [H, C], f16, isOutput=False)
    probs_d = nc.declare_dram_parameter("probsT", [C, S, BC], f32, isOutput=True)

    with tile.TileContext(nc) as tc:
        import contextlib
        ctx = contextlib.ExitStack()
        with ctx:
            singles = ctx.enter_context(tc.tile_pool(name="singles", bufs=1))
            psA = ctx.enter_context(tc.tile_pool(name="psA", bufs=1, space="PSUM"))
            psB = ctx.enter_context(tc.tile_pool(name="psB", bufs=1, space="PSUM"))

            # ---- persistent SBUF state ----
            h_nat = singles.tile([BC, T, D], f16)
            projT = singles.tile([128, KT, BC, T], f16)  # [h', m, b, t]
            hidT = singles.tile([128, KT, S + 1, BC], f16)  # h states, slot 0 = 0
            oneh = singles.tile([C, S, BC], f16)
            wi2h = singles.tile([128, KT, H], f16)
            wh2h = singles.tile([128, KT, H], f16)
            bh2hT = singles.tile([128, KT, 1], f32)
            wsc = singles.tile([128, KT, 32], f16)
            wxc = singles.tile([128, KT, 4 * H], f16)
            wxo = singles.tile([C, 4 * H], f16)
            wh = singles.tile([128, KT, 4 * H], f16)
            wg = singles.tile([128, KT, C], f16)
            ident = singles.tile([128, 128], f16)
            cT = singles.tile([BC, H], f16)

            # ---- load everything ----
            nc.sync.dma_start(out=h_nat, in_=h_nat_d[:])
            nc.sync.dma_start(out=oneh, in_=oneh_d[:])
            nc.sync.dma_start(
                out=wi2h, in_=wi2h_d[:].rearrange("(k p) h -> p k h", p=128))
            nc.sync.dma_start(
                out=wh2h, in_=wh2h_d[:].rearrange("(k p) h -> p k h", p=128))
            nc.sync.dma_start(
                out=bh2hT, in_=bh2h_d[:].rearrange("(k p) o -> p k o", p=128))
            nc.sync.dma_start(
                out=wsc, in_=wsc_d[:].rearrange("(k p) o -> p k o", p=128))
            nc.sync.dma_start(
                out=wxc, in_=wxc_d[:].rearrange("(k p) g -> p k g", p=128))
            nc.sync.dma_start(out=wxo, in_=wxo_d[:])
            nc.sync.dma_start(
                out=wh, in_=wh_d[:].rearrange("(k p) g -> p k g", p=128))
            nc.sync.dma_start(
                out=wg, in_=wg_d[:].rearrange("(k p) c -> p k c", p=128))
            make_identity(nc, ident)
            nc.vector.memset(hidT[:, :, 0, :], 0.0)
            nc.vector.memset(cT, 0.0)

            # ---- precompute projT = (batch_H @ Wi2h)^T : [h', m, (b t)] ----
            projT_f = projT[:].rearrange("p m b t -> p m (b t)")
            with tc.tile_pool(name="ht", bufs=1) as ht_pool:
                h_tt = ht_pool.tile([128, KT, BC, T], f16)
                nc.sync.dma_start(
                    out=h_tt,
                    in_=h_t_d[:].rearrange("(k p) b t -> p k b t", p=128))
                h_tt_f = h_tt[:].rearrange("p k b t -> p k (b t)")
                for m in range(KT):
                    for c in range(TB // NCHUNK):
                        ps = psA.tile([128, 4, NCHUNK], f32, tag="e")
                        sl = slice(c * NCHUNK, (c + 1) * NCHUNK)
                        for k in range(KT):
                            nc.tensor.matmul(
                                ps[:, 0, :], wi2h[:, k, m * 128:(m + 1) * 128],
                                h_tt_f[:, k, sl], start=(k == 0),
                                stop=(k == KT - 1))
                        eng = nc.scalar if (c % 2 == 0) else nc.vector
                        if eng is nc.scalar:
                            eng.copy(out=projT_f[:, m, sl], in_=ps[:, 0, :])
                        else:
                            eng.tensor_copy(projT_f[:, m, sl], ps[:, 0, :])

            work = ctx.enter_context(tc.tile_pool(name="work", bufs=2))
            small = ctx.enter_context(tc.tile_pool(name="small", bufs=2))
            big1 = ctx.enter_context(tc.tile_pool(name="big1", bufs=1))
            ahp = ctx.enter_context(tc.tile_pool(name="ahp", bufs=16))

            # ---- decode steps ----
            for s in range(S):
                # hp = h @ Wh2h + bh2h, transposed [h', m, b]
                ps_hp = psB.tile([BC, KT, 128], f32, tag="misc")
                for m in range(KT):
                    for k in range(KT):
                        nc.tensor.matmul(
                            ps_hp[:, m, :], wh2h[:, k, m * 128:(m + 1) * 128],
                            hidT[:, k, s, :], start=(k == 0), stop=(k == KT - 1))
                hpT = small.tile([128, KT, 128], f16)
                for m in range(KT):
                    nc.vector.tensor_scalar(
                        out=hpT[:, m, :], in0=ps_hp[:, m, :],
                        scalar1=bh2hT[:, m, :], scalar2=None, op0=ALU.add)
                # replicate hp 8x in an inner dim so the z-add's broadcast AP
                # ends with a step-1 dim (keeps DVE 2x mode); per-m so the
                # z-pipeline fills sooner
                hpR = small.tile([128, KT, 128, 8], f16, tag="hpR")
                for m in range(KT):
                    base = hpT[:, m, :]
                    nc.vector.tensor_copy(
                        hpR[:, m, :, :],
                        bass.AP(tensor=base.tensor, offset=base.offset,
                                ap=[base.ap[0], [1, 128], [0, 8]]))

                # z = projT + hp (bcast over t); tanh; e = w . tanh
                ps_e = psA.tile([128, 4, NCHUNK], f32, tag="e")
                for m in range(KT):
                    cg = 0
                    b0 = 0
                    for nbz in (8, 24, 32, 64):
                        bsl = slice(b0, b0 + nbz)
                        gp = (m == KT - 1 and b0 == 64)
                        # the last chunk of m=1 runs on gpsimd (plain TT add)
                        # from a dedicated tile so it isn't starved by the
                        # z-slot rotation
                        z = work.tile([128, 64, T], f16,
                                      tag=("zgp" if gp else "z"))
                        hb = hpR[:, m, b0:, :]
                        zeng = nc.vector  # gpsimd TT crashed HW (NRT 101)
                        zeng.tensor_tensor(
                            out=z[:, :nbz, :].rearrange(
                                "p b (r i) -> p b r i", i=8),
                            in0=projT[:, m, bsl, :].rearrange(
                                "p b (r i) -> p b r i", i=8),
                            in1=bass.AP(tensor=hb.tensor, offset=hb.offset,
                                        ap=[hb.ap[0], [hb.ap[1][0], nbz],
                                            [0, T // 8], [1, 8]]),
                            op=ALU.add)
                        th = work.tile([128, 64 * T], f16, tag="th")
                        nc.scalar.activation(
                            out=th[:, :nbz * T],
                            in_=z[:, :nbz, :].rearrange("p b t -> p (b t)"),
                            func=AF.Tanh)
                        for q in range(nbz * T // NCHUNK):
                            col = 32 * (cg // 4)
                            bank = cg % 4
                            nc.tensor.matmul(
                                ps_e[col:col + 32, bank, :], wsc[:, m, :],
                                th[:, q * NCHUNK:(q + 1) * NCHUNK],
                                start=(m == 0), stop=(m == KT - 1),
                                tile_position=(0, col),
                                skip_group_check=True)
                            cg += 1
                        b0 += nbz
                    assert b0 == BC

                # softmax over t (e bounded: no max subtraction needed).
                # Chunk cg lives replicated on partitions
                # [32*(cg//6), +32), bank cg%6. Valid rectangles:
                # [0:96, banks 0:4] and [0:64, banks 4:6].
                expa = big1.tile([128, 4, NCHUNK], f32, tag="expa")
                nc.scalar.activation(out=expa[0:128, 0:4, :],
                                     in_=ps_e[0:128, 0:4, :], func=AF.Exp)
                # unnormalized alpha scatter -> [128 b, 64 t]; chunk cg is
                # on partition-row 32*(cg//6), bank cg%6, so rows 0/32 cover
                # chunks 0..11 (-> b 0..96) and row 64 banks 0..3 -> b 96..128
                alpha = small.tile([BC, T], f32, tag="alpha")
                qeng = (nc.sync, nc.scalar, nc.sync, nc.scalar)
                for row in range(4):
                    er = expa[32 * row:32 * row + 1, 0:4, :]
                    qeng[row].dma_start(
                        out=alpha[32 * row:32 * row + 32, :],
                        in_=bass.AP(tensor=er.tensor, offset=er.offset,
                                    ap=[er.ap[0], [NCHUNK, 4], [1, NCHUNK]]))
                # Z = sum_t exp in [b, t] layout; rzb = 1/Z  [128, 1]
                zsum = small.tile([BC, 1], f32, tag="zsum")
                nc.vector.reduce_sum(out=zsum, in_=alpha,
                                     axis=mybir.AxisListType.X)
                rzb = small.tile([BC, 1], f32, tag="rzb")
                nc.vector.reciprocal(out=rzb, in_=zsum)
                if s == 0:
                    _CACHE.setdefault("dbg", {})["alpha0"] = alpha.tensor.name
                    _CACHE["dbg"]["expa0"] = expa.tensor.name
                    _CACHE["dbg"]["hpT0"] = hpT.tensor.name
                # context = sum_t alpha~[:, t] * H[:, t, :] (unnormalized):
                # DVE makes alpha_t*H_t slices at 4x, PE accumulates them into
                # PSUM (fp32) via identity matmuls; the final copy rescales by
                # rzb = 1/Z.
                ps_cx = psB.tile([BC, D], f32, tag="misc")
                ahs = []
                for t in range(T):
                    ah = ahp.tile([BC, D], f16, tag="ah")
                    nc.vector.tensor_scalar(
                        out=ah, in0=h_nat[:, t, :], scalar1=alpha[:, t:t + 1],
                        scalar2=None, op0=ALU.mult)
                    ahs.append(ah)
                    if t % 16 == 15:
                        for j, a in enumerate(ahs):
                            tt = t - len(ahs) + 1 + j
                            nc.tensor.matmul(ps_cx, ident, a, start=(tt == 0),
                                             stop=(tt == T - 1))
                        ahs = []
                # gates partial (h and onehot contributions don't need ctx)
                ps_g = psA.tile([BC, 4 * H], f32, tag="e")
                for nchk in range(2):
                    nsl = slice(nchk * NCHUNK, (nchk + 1) * NCHUNK)
                    for k in range(KT):
                        nc.tensor.matmul(
                            ps_g[:, nsl], hidT[:, k, s, :], wh[:, k, nsl],
                            start=(k == 0), stop=False)
                    nc.tensor.matmul(
                        ps_g[:, nsl], oneh[:, s, :], wxo[:, nsl],
                        start=False, stop=False)
                ctxv = small.tile([BC, D], f16, tag="ctxv")
                nc.vector.tensor_scalar(
                    out=ctxv, in0=ps_cx, scalar1=rzb[:, 0:1], scalar2=None,
                    op0=ALU.mult)
                if s == 0:
                    _CACHE["dbg"]["ctx0"] = ctxv.tensor.name
                    _CACHE["dbg"]["projT"] = projT.tensor.name
                    _CACHE["dbg"]["hidT"] = hidT.tensor.name

                # transpose context -> [d', m, b]
                ps_ct = psB.tile([BC, KT, 128], f16, tag="misc")
                ctxT = small.tile([128, KT, 128], f16, tag="ctxT")
                for m in range(KT):
                    nc.tensor.transpose(
                        ps_ct[:, m, :], ctxv[:, m * 128:(m + 1) * 128], ident)
                    nc.vector.tensor_copy(ctxT[:, m, :], ps_ct[:, m, :])

                if s == 0:
                    _CACHE["dbg"]["ctxT0"] = ctxT.tensor.name
                # gates[b, 4H] = h.Wh + onehot.Wxo + ctx.Wxc with the
                # activations as the stationary operand (5 weight loads, 10
                # N=512 matmuls) instead of 40 small ones.
                for nchk in range(2):
                    nsl = slice(nchk * NCHUNK, (nchk + 1) * NCHUNK)
                    for k in range(KT):
                        nc.tensor.matmul(
                            ps_g[:, nsl], ctxT[:, k, :], wxc[:, k, nsl],
                            start=False, stop=(k == KT - 1))

                # LSTM pointwise in [b, g] layout. keras order i, f, g, o
                # (b_lstm folded into wxo host-side).
                sig_if = small.tile([BC, 2 * H], f16, tag="sig_if")
                tg = small.tile([BC, H], f16, tag="tg")
                sig_o = small.tile([BC, H], f16, tag="sig_o")
                nc.scalar.activation(out=sig_if, in_=ps_g[:, 0:2 * H],
                                     func=AF.Sigmoid)
                nc.scalar.activation(out=tg, in_=ps_g[:, 2 * H:3 * H],
                                     func=AF.Tanh)
                nc.scalar.activation(out=sig_o, in_=ps_g[:, 3 * H:4 * H],
                                     func=AF.Sigmoid)
                t1 = small.tile([BC, H], f16, tag="t1")
                t2 = small.tile([BC, H], f16, tag="t2")
                nc.vector.tensor_tensor(out=t1, in0=sig_if[:, H:2 * H],
                                        in1=cT, op=ALU.mult)
                nc.vector.tensor_tensor(out=t2, in0=sig_if[:, 0:H], in1=tg,
                                        op=ALU.mult)
                nc.vector.tensor_tensor(out=cT, in0=t1, in1=t2, op=ALU.add)
                tc_t = small.tile([BC, H], f16, tag="tc_t")
                nc.scalar.activation(out=tc_t, in_=cT, func=AF.Tanh)
                h_bd = small.tile([BC, H], f16, tag="h_bd")
                nc.vector.tensor_tensor(out=h_bd, in0=sig_o, in1=tc_t,
                                        op=ALU.mult)
                # transpose h back to [h', k, b] for the next step's matmuls
                ps_h = psB.tile([BC, KT, 128], f16, tag="misc")
                for k in range(KT):
                    nc.tensor.transpose(
                        ps_h[:, k, :], h_bd[:, k * 128:(k + 1) * 128], ident)
                    nc.vector.tensor_copy(hidT[:, k, s + 1, :], ps_h[:, k, :])

            # ---- generator: probsT = Wg^T . h_s for all steps ----
            hid_f = hidT[:].rearrange("p k s b -> p k (s b)")
            NS = S * BC  # 3328
            probs_sb = singles.tile([C, S, BC], f32)
            probs_f = probs_sb[:].rearrange("c s b -> c (s b)")
            pos = 0
            ci = 0
            while pos < NS:
                n = min(NCHUNK, NS - pos)
                ps_p = psB.tile([C, NCHUNK], f32, tag="misc")
                for k in range(KT):
                    nc.tensor.matmul(
                        ps_p[:, :n], wg[:, k, :], hid_f[:, k, BC + pos:BC + pos + n],
                        start=(k == 0), stop=(k == KT - 1))
                eng = nc.scalar if (ci % 2 == 0) else nc.vector
                if eng is nc.scalar:
                    eng.copy(out=probs_f[:, pos:pos + n], in_=ps_p[:, :n])
                else:
                    eng.tensor_copy(probs_f[:, pos:pos + n], ps_p[:, :n])
                pos += n
                ci += 1
            nc.sync.dma_start(out=probs_d[:], in_=probs_sb)

    _split_excess_waits(nc)
    return nc


def _get_module():
    if "nc" not in _CACHE:
        _CACHE["nc"] = _build()
    return _CACHE["nc"]


def build_in_maps(batch_H, text, batch_max_length, Wi2h, Wh2h, bh2h, w_score,
                  Wx, Wh, b_lstm, Wg, bg):
    batch_H = np.asarray(batch_H, dtype=np.float32)
    text = np.asarray(text)
    assert int(batch_max_length) + 1 == S
    assert batch_H.shape == (B, T, D)

    f16 = np.float16
    bh16 = batch_H.astype(f16)
    # one-hot text: [B, S, C] -> per-core [S, C, BC]
    oh = (text[:, :S, None] == np.arange(C)[None, None, :])

    Wx = np.asarray(Wx, np.float32)
    wxo_p = (Wx[D:D + C, :] + np.asarray(b_lstm, np.float32)[None, :]).astype(f16)
    weights = {
        "wi2h": np.ascontiguousarray(np.asarray(Wi2h, np.float32).astype(f16)),
        "wh2h": np.ascontiguousarray(np.asarray(Wh2h, np.float32).astype(f16)),
        "bh2hT": np.ascontiguousarray(
            np.asarray(bh2h, np.float32).reshape(H, 1)),
        "wsc": np.ascontiguousarray(np.tile(
            np.asarray(w_score, np.float32).reshape(H, 1), (1, 32)).astype(f16)),
        "wxc": np.ascontiguousarray(Wx[:D, :].astype(f16)),
        "wxo": np.ascontiguousarray(wxo_p),
        "wh": np.ascontiguousarray(np.asarray(Wh, np.float32).astype(f16)),
        "wg": np.ascontiguousarray(np.asarray(Wg, np.float32).astype(f16)),
    }

    in_maps = []
    for c in range(NCORES):
        bsl = slice(c * BC, (c + 1) * BC)
        in_maps.append({
            "h_nat": np.ascontiguousarray(bh16[bsl]),
            "h_t": np.ascontiguousarray(bh16[bsl].transpose(2, 0, 1)),
            "onehot": np.ascontiguousarray(
                oh[bsl].transpose(2, 1, 0).astype(f16)),
            **weights,
        })
    return in_maps


def kernel(**inputs):
    in_maps = build_in_maps(**inputs)
    bg = inputs["bg"]

    nc = _get_module()
    res = run_bass_kernel_spmd(nc, in_maps, list(range(NCORES)))

    out = np.empty((B, S, C), np.float32)
    for c in range(NCORES):
        out[c * BC:(c + 1) * BC] = res.results[c]["probsT"].transpose(2, 1, 0)
    out += np.asarray(bg, np.float32)[None, None, :]
    return out


if __name__ == "__main__":
    _build()
    print("build OK")
